# revision 9
# baseline (speedup 1.0000x reference)
"""GCNConv on 8 Trainium2 NeuronCores.

Computes out = D^-1/2 (A+I) D^-1/2 (x @ W.T + b) for a dense adjacency
A [16384, 16384], x [16384, 512], W [256, 512], b [256].

Strategy (1D row partition over 8 cores, 2048 rows each):
 - Host prep per core m: upload AT_m = (A[rows_m, :] + I)^T as bf16
   [16384, 2048] (partition-natural layout for the PE contraction over j),
   x^T shard fp32, W^T fp32, b replicated.
 - Phase 1 (device): rowsums s_i = sum_j (A+I)[i, j] via ones^T @ AT
   matmuls on the tensor engine; h = x @ W.T + b in fp32 on the PE;
   d = 1/sqrt(s) (ACT sqrt + DVE reciprocal); g = d * h cast to bf16.
 - AllGather g across the 8 cores (2048x256 bf16 per rank -> 16384x256).
 - Phase 2 (device): out_m = d * (AT_m^T @ g) with PSUM accumulation over
   all 128 j-tiles; row-scale by d; write out [2048, 256] fp32.
"""

import numpy as np
import ml_dtypes

N = 16384
IN_DIM = 512
OUT_DIM = 256
NCORES = 8
ROWS = N // NCORES  # 2048

BF16 = ml_dtypes.bfloat16

# Most recent BassKernelResults (for test harnesses that want exec_time_ns).
LAST_RESULTS = None

_PROGRAM_CACHE = {}


def build_program(n_total, rows, in_dim, out_dim, n_cores, slab_bufs=8):
    """Build + compile the per-core Bass program (same NEFF on all cores)."""
    import concourse.bacc as bacc
    import concourse.mybir as mybir
    import concourse.tile as tile

    f32 = mybir.dt.float32
    bf16 = mybir.dt.bfloat16
    P = 128

    n_jt = n_total // P            # tiles along the contraction axis j
    n_it = rows // P               # output row tiles per core
    rs_chunk = min(512, rows)      # rowsum moving free size
    n_rs = rows // rs_chunk        # rowsum chunks (psum banks used)
    n_ct = in_dim // P             # contraction tiles for the h GEMM
    assert n_rs <= 4 and n_it <= 16

    nc = bacc.Bacc(
        "TRN2",
        target_bir_lowering=False,
        debug=False,
        enable_asserts=False,
        num_devices=n_cores,
    )

    at_d = nc.dram_tensor("at", [n_total, rows], bf16, kind="ExternalInput").ap()
    xt_d = nc.dram_tensor("xt", [in_dim, rows], f32, kind="ExternalInput").ap()
    wt_d = nc.dram_tensor("wt", [in_dim, out_dim], f32, kind="ExternalInput").ap()
    brep_d = nc.dram_tensor("brep", [P, out_dim], f32, kind="ExternalInput").ap()
    out_d = nc.dram_tensor("out", [rows, out_dim], f32, kind="ExternalOutput").ap()

    with tile.TileContext(nc) as tc:
        with (
            tc.tile_pool(name="const", bufs=1) as cpool,
            tc.tile_pool(name="slab", bufs=slab_bufs) as slab_pool,
            tc.tile_pool(name="small", bufs=2) as spool,
            tc.tile_pool(name="psum", bufs=1, space="PSUM") as pspool,
            tc.tile_pool(name="dram", bufs=1, space="DRAM") as dram_pool,
        ):
            ones_sb = cpool.tile([P, 1], bf16)
            nc.vector.memset(ones_sb[:], 1.0)
            zeros_sb = cpool.tile([P, P], bf16)
            nc.vector.memset(zeros_sb[:], 0.0)

            # x^T staged as [c, ct*rows + i]; W^T as [c, ct*out_dim + k]
            xt_sb = cpool.tile([P, n_ct * rows], f32)
            nc.sync.dma_start(
                xt_sb[:].rearrange("c (ct i) -> c ct i", ct=n_ct),
                xt_d.rearrange("(ct c) i -> c ct i", c=P),
            )
            wt_sb = cpool.tile([P, n_ct * out_dim], f32)
            nc.sync.dma_start(
                wt_sb[:].rearrange("c (ct k) -> c ct k", ct=n_ct),
                wt_d.rearrange("(ct c) k -> c ct k", c=P),
            )
            brep_sb = cpool.tile([P, out_dim], f32)
            nc.sync.dma_start(brep_sb[:], brep_d[:, :])

            # bf16 h for the whole graph, laid out [p, jt*out_dim + k]
            hall_sb = cpool.tile([P, n_jt * out_dim], bf16)

            h_local_dr = dram_pool.tile([rows, out_dim], bf16)
            h_all_dr = dram_pool.tile([n_total, out_dim], bf16, addr_space="Shared")
            s_dr = dram_pool.tile([1, rows], f32)
            d_local_dr = dram_pool.tile([1, rows], f32)
            d_all_dr = dram_pool.tile([1, n_total], f32, addr_space="Shared")

            # ---- Phase 1a: h = x @ W.T + b, AllGather h early (hidden) ----
            h_sb = cpool.tile([P, n_it * out_dim], f32)
            for it in range(n_it):
                h_ps = pspool.tile(
                    [P, out_dim], f32, name="h_ps", tag=f"bank{n_rs + it % 2}"
                )
                for ct in range(n_ct):
                    nc.tensor.matmul(
                        h_ps[:, :],
                        xt_sb[:, ct * rows + it * P : ct * rows + (it + 1) * P],
                        wt_sb[:, ct * out_dim : (ct + 1) * out_dim],
                        start=(ct == 0),
                        stop=(ct == n_ct - 1),
                    )
                nc.vector.tensor_add(
                    h_sb[:, it * out_dim : (it + 1) * out_dim], h_ps[:, :], brep_sb[:]
                )
                hb_t = spool.tile([P, out_dim], bf16, name="hb_t", tag="hbt", bufs=3)
                nc.vector.tensor_copy(
                    hb_t[:], h_sb[:, it * out_dim : (it + 1) * out_dim]
                )
                nc.sync.dma_start(h_local_dr[it * P : (it + 1) * P, :], hb_t[:])

            nc.gpsimd.collective_compute(
                "AllGather",
                mybir.AluOpType.bypass,
                replica_groups=[list(range(n_cores))],
                ins=[h_local_dr[:, :]],
                outs=[h_all_dr[:, :]],
            )
            nc.sync.dma_start(
                hall_sb[:].rearrange("p (jt k) -> p jt k", jt=n_jt),
                h_all_dr[:, :].rearrange("(jt p) k -> p jt k", p=P),
            )

            # ---- Phase 1b: rowsums of (A+I) via ones^T @ AT ----
            s_ps = [
                pspool.tile([1, rs_chunk], f32, name=f"s_ps{c}", tag=f"bank{c}")
                for c in range(n_rs)
            ]
            for jt in range(n_jt):
                slab = slab_pool.tile([P, rows], bf16, name="slab", tag="slab")
                nc.sync.dma_start(slab[:], at_d[jt * P : (jt + 1) * P, :])
                for c in range(n_rs):
                    nc.tensor.matmul(
                        s_ps[c][:, :],
                        ones_sb[:],
                        slab[:, c * rs_chunk : (c + 1) * rs_chunk],
                        start=(jt == 0),
                        stop=(jt == n_jt - 1),
                    )

            # s -> SBUF -> DRAM -> per-partition [p, t] layout
            s_sb = spool.tile([1, rows], f32)
            for c in range(n_rs):
                nc.scalar.copy(
                    s_sb[:, c * rs_chunk : (c + 1) * rs_chunk], s_ps[c][:, :]
                )
            nc.sync.dma_start(s_dr[:, :], s_sb[:])
            sT_sb = spool.tile([P, n_it], f32)
            nc.sync.dma_start(sT_sb[:], s_dr[0, :].rearrange("(t p) -> p t", p=P))

            # d = 1/sqrt(s)  (the +1 self-loop is already folded into AT's diag)
            sq_sb = spool.tile([P, n_it], f32)
            nc.scalar.sqrt(sq_sb[:], sT_sb[:])
            d_col = cpool.tile([P, n_it], f32)
            nc.vector.reciprocal(d_col[:], sq_sb[:])

            # AllGather d (tiny): d_col -> DRAM [rows] -> AG -> [n_total] -> [p, jt]
            nc.sync.dma_start(
                d_local_dr[0, :].rearrange("(t p) -> p t", p=P), d_col[:]
            )
            nc.gpsimd.collective_compute(
                "AllGather",
                mybir.AluOpType.bypass,
                replica_groups=[list(range(n_cores))],
                ins=[d_local_dr[:, :]],
                outs=[d_all_dr[:, :]],
            )
            d_j = cpool.tile([P, n_jt], f32)
            nc.sync.dma_start(d_j[:], d_all_dr[0, :].rearrange("(jt p) -> p jt", p=P))

            # ---- Phase 2: out = d * (AT^T @ g) ----
            # Two [*, out_dim] accumulation chains share each PSUM bank.
            # Measured HW semantics (psum_probe.py): start=False accumulates
            # unconditionally; start=True overwrites only the region it writes.
            # So zero each whole bank once with a zero-stationary matmul, then
            # run both chains with start=False throughout.
            n_acc = (n_it + 1) // 2
            mm_ps = [
                pspool.tile([P, 2 * out_dim], f32, name=f"mm{q}", tag=f"bank{q}")
                for q in range(n_acc)
            ]
            for q in range(n_acc):
                nc.tensor.matmul(
                    mm_ps[q][:, :],
                    zeros_sb[:],
                    hall_sb[:, : 2 * out_dim],
                    start=True,
                    stop=True,
                    skip_group_check=True,
                )
            for jt in range(n_jt):
                slab = slab_pool.tile([P, rows], bf16, name="slab2", tag="slab")
                nc.sync.dma_start(slab[:], at_d[jt * P : (jt + 1) * P, :])
                # g[jt] = d[jt] * h[jt]  on the (otherwise idle) vector engine
                g_t = spool.tile([P, out_dim], bf16, name="g_t", tag="gt", bufs=6)
                nc.vector.tensor_scalar(
                    g_t[:],
                    hall_sb[:, jt * out_dim : (jt + 1) * out_dim],
                    d_j[:, jt : jt + 1],
                    None,
                    op0=mybir.AluOpType.mult,
                )
                for it in range(n_it):
                    nc.tensor.matmul(
                        mm_ps[it // 2][:, (it % 2) * out_dim : (it % 2 + 1) * out_dim],
                        slab[:, it * P : (it + 1) * P],
                        g_t[:],
                        start=False,
                        stop=(jt == n_jt - 1),
                        skip_group_check=True,
                    )
            for it in range(n_it):
                o_t = spool.tile([P, out_dim], f32, name="o_t", tag="ot", bufs=3)
                nc.vector.tensor_scalar(
                    o_t[:],
                    mm_ps[it // 2][:, (it % 2) * out_dim : (it % 2 + 1) * out_dim],
                    d_col[:, it : it + 1],
                    None,
                    op0=mybir.AluOpType.mult,
                )
                nc.sync.dma_start(out_d[it * P : (it + 1) * P, :], o_t[:])

    nc.compile()
    return nc


def _get_program(key):
    if key not in _PROGRAM_CACHE:
        _PROGRAM_CACHE[key] = build_program(*key)
    return _PROGRAM_CACHE[key]


def make_in_maps(x, adj_mat, W, b, n_total, rows, n_cores):
    """Shard + lay out inputs per core: AT bf16 with +I folded, x^T, W^T, b."""
    x = np.asarray(x, dtype=np.float32)
    adj = np.asarray(adj_mat, dtype=np.float32)
    W = np.asarray(W, dtype=np.float32)
    b = np.asarray(b, dtype=np.float32)

    WT = np.ascontiguousarray(W.T)
    brep = np.ascontiguousarray(np.broadcast_to(b[None, :], (128, W.shape[0])))
    il = np.arange(rows)
    in_maps = []
    for m in range(n_cores):
        sl = slice(m * rows, (m + 1) * rows)
        at_bf = np.ascontiguousarray(adj[sl, :].astype(BF16).T)  # [n_total, rows]
        diag = adj[m * rows + il, m * rows + il] + np.float32(1.0)
        at_bf[m * rows + il, il] = diag.astype(BF16)
        xt = np.ascontiguousarray(x[sl, :].T)
        in_maps.append({"at": at_bf, "xt": xt, "wt": WT, "brep": brep})
    return in_maps


def run_sharded(x, adj_mat, W, b, n_total, in_dim, out_dim, n_cores, **run_kwargs):
    global LAST_RESULTS
    from concourse.bass_utils import run_bass_kernel_spmd

    rows = n_total // n_cores
    nc = _get_program((n_total, rows, in_dim, out_dim, n_cores))
    in_maps = make_in_maps(x, adj_mat, W, b, n_total, rows, n_cores)
    res = run_bass_kernel_spmd(nc, in_maps, core_ids=list(range(n_cores)), **run_kwargs)
    LAST_RESULTS = res
    return np.concatenate([r["out"] for r in res.results], axis=0)


def kernel(x=None, adj_mat=None, W=None, b=None, **_ignored):
    out = run_sharded(x, adj_mat, W, b, N, IN_DIM, OUT_DIM, NCORES)
    return np.asarray(out, dtype=np.float32)


# revision 17
# speedup vs baseline: 1.0907x; 1.0907x over previous
"""GCNConv on 8 Trainium2 NeuronCores.

Computes out = D^-1/2 (A+I) D^-1/2 (x @ W.T + b) for a dense adjacency
A [16384, 16384], x [16384, 512], W [256, 512], b [256].

Strategy (1D row partition over 8 cores, 2048 rows each):
 - Host prep per core m: upload AT_m = (A[rows_m, :] + I)^T as bf16
   [16384, 2048] (partition-natural layout for the PE contraction over j),
   x^T shard fp32, W^T fp32, b replicated.
 - Phase 1 (device): rowsums s_i = sum_j (A+I)[i, j] via ones^T @ AT
   matmuls on the tensor engine; h = x @ W.T + b in fp32 on the PE;
   d = 1/sqrt(s) (ACT sqrt + DVE reciprocal); g = d * h cast to bf16.
 - AllGather g across the 8 cores (2048x256 bf16 per rank -> 16384x256).
 - Phase 2 (device): out_m = d * (AT_m^T @ g) with PSUM accumulation over
   all 128 j-tiles; row-scale by d; write out [2048, 256] fp32.
"""

import numpy as np
import ml_dtypes

N = 16384
IN_DIM = 512
OUT_DIM = 256
NCORES = 8
ROWS = N // NCORES  # 2048

BF16 = ml_dtypes.bfloat16

# Most recent BassKernelResults (for test harnesses that want exec_time_ns).
LAST_RESULTS = None

_PROGRAM_CACHE = {}


def build_program(n_total, rows, in_dim, out_dim, n_cores, slab_bufs=12):
    """Build + compile the per-core Bass program (same NEFF on all cores)."""
    import concourse.bacc as bacc
    import concourse.mybir as mybir
    import concourse.tile as tile

    f32 = mybir.dt.float32
    bf16 = mybir.dt.bfloat16
    P = 128

    n_jt = n_total // P            # tiles along the contraction axis j
    n_it = rows // P               # output row tiles per core
    rs_chunk = min(512, rows)      # rowsum moving free size
    n_rs = rows // rs_chunk        # rowsum chunks (psum banks used)
    n_ct = in_dim // P             # contraction tiles for the h GEMM
    assert n_rs <= 4 and n_it <= 16

    nc = bacc.Bacc(
        "TRN2",
        target_bir_lowering=False,
        debug=False,
        enable_asserts=False,
        num_devices=n_cores,
    )

    at_d = nc.dram_tensor("at", [n_total, rows], bf16, kind="ExternalInput").ap()
    xt_d = nc.dram_tensor("xt", [in_dim, rows], f32, kind="ExternalInput").ap()
    wt_d = nc.dram_tensor("wt", [in_dim, out_dim], f32, kind="ExternalInput").ap()
    brep_d = nc.dram_tensor("brep", [P, out_dim], f32, kind="ExternalInput").ap()
    out_d = nc.dram_tensor("out", [rows, out_dim], f32, kind="ExternalOutput").ap()

    with tile.TileContext(nc) as tc:
        with (
            tc.tile_pool(name="const", bufs=1) as cpool,
            tc.tile_pool(name="slab", bufs=slab_bufs) as slab_pool,
            tc.tile_pool(name="small", bufs=2) as spool,
            tc.tile_pool(name="psum", bufs=1, space="PSUM") as pspool,
            tc.tile_pool(name="dram", bufs=1, space="DRAM") as dram_pool,
        ):
            ones_sb = cpool.tile([P, 1], bf16)
            nc.vector.memset(ones_sb[:], 1.0)
            zeros_sb = cpool.tile([P, P], bf16)
            nc.vector.memset(zeros_sb[:], 0.0)

            # x^T staged as [c, ct*rows + i]; W^T as [c, ct*out_dim + k]
            xt_sb = cpool.tile([P, n_ct * rows], f32)
            nc.gpsimd.dma_start(
                xt_sb[:].rearrange("c (ct i) -> c ct i", ct=n_ct),
                xt_d.rearrange("(ct c) i -> c ct i", c=P),
            )
            wt_sb = cpool.tile([P, n_ct * out_dim], f32)
            nc.gpsimd.dma_start(
                wt_sb[:].rearrange("c (ct k) -> c ct k", ct=n_ct),
                wt_d.rearrange("(ct c) k -> c ct k", c=P),
            )
            brep_sb = cpool.tile([P, out_dim], f32)
            nc.gpsimd.dma_start(brep_sb[:], brep_d[:, :])

            # bf16 h for the whole graph, laid out [p, jt*out_dim + k]
            hall_sb = cpool.tile([P, n_jt * out_dim], bf16)

            h_local_dr = dram_pool.tile([rows, out_dim], bf16)
            h_all_dr = dram_pool.tile([n_total, out_dim], bf16, addr_space="Shared")
            s_dr = dram_pool.tile([1, rows], f32)
            d_local_dr = dram_pool.tile([P, n_it], f32)
            d_all_dr = dram_pool.tile([n_cores, P * n_it], f32, addr_space="Shared")

            # ---- Phase 1a: h = x @ W.T + b, AllGather h early (hidden) ----
            h_sb = cpool.tile([P, n_it * out_dim], f32)
            for it in range(n_it):
                h_ps = pspool.tile(
                    [P, out_dim], f32, name="h_ps", tag=f"bank{n_rs + it % 2}"
                )
                for ct in range(n_ct):
                    nc.tensor.matmul(
                        h_ps[:, :],
                        xt_sb[:, ct * rows + it * P : ct * rows + (it + 1) * P],
                        wt_sb[:, ct * out_dim : (ct + 1) * out_dim],
                        start=(ct == 0),
                        stop=(ct == n_ct - 1),
                    )
                nc.vector.tensor_add(
                    h_sb[:, it * out_dim : (it + 1) * out_dim], h_ps[:, :], brep_sb[:]
                )
                hb_t = spool.tile([P, out_dim], bf16, name="hb_t", tag="hbt", bufs=3)
                nc.vector.tensor_copy(
                    hb_t[:], h_sb[:, it * out_dim : (it + 1) * out_dim]
                )
                nc.gpsimd.dma_start(h_local_dr[it * P : (it + 1) * P, :], hb_t[:])

            nc.gpsimd.collective_compute(
                "AllGather",
                mybir.AluOpType.bypass,
                replica_groups=[list(range(n_cores))],
                ins=[h_local_dr[:, :]],
                outs=[h_all_dr[:, :]],
            )

            # ---- Phase 1b: rowsums of (A+I) via ones^T @ AT ----
            s_ps = [
                pspool.tile([1, rs_chunk], f32, name=f"s_ps{c}", tag=f"bank{c}")
                for c in range(n_rs)
            ]
            for jt in range(n_jt):
                slab = slab_pool.tile([P, rows], bf16, name="slab", tag="slab")
                nc.sync.dma_start(slab[:], at_d[jt * P : (jt + 1) * P, :])
                for c in range(n_rs):
                    nc.tensor.matmul(
                        s_ps[c][:, :],
                        ones_sb[:],
                        slab[:, c * rs_chunk : (c + 1) * rs_chunk],
                        start=(jt == 0),
                        stop=(jt == n_jt - 1),
                    )

            # h_all -> SBUF, on the gpsimd (SWDGE) queues so it doesn't
            # head-of-line-block the slab stream while waiting on the AG.
            # Chunked to stay under the per-DMA descriptor limit.
            hall_chunks = max(1, n_jt // 16)
            cjt = n_jt // hall_chunks
            for hc in range(hall_chunks):
                nc.gpsimd.dma_start(
                    hall_sb[:, hc * cjt * out_dim : (hc + 1) * cjt * out_dim]
                    .rearrange("p (jt k) -> p jt k", jt=cjt),
                    h_all_dr[hc * cjt * P : (hc + 1) * cjt * P, :]
                    .rearrange("(jt p) k -> p jt k", p=P),
                )


            # s -> SBUF -> DRAM -> per-partition [p, t] layout
            s_sb = spool.tile([1, rows], f32)
            for c in range(n_rs):
                nc.scalar.copy(
                    s_sb[:, c * rs_chunk : (c + 1) * rs_chunk], s_ps[c][:, :]
                )
            nc.sync.dma_start(s_dr[:, :], s_sb[:])
            sT_sb = spool.tile([P, n_it], f32)
            nc.sync.dma_start(sT_sb[:], s_dr[0, :].rearrange("(t p) -> p t", p=P))

            # d = 1/sqrt(s)  (the +1 self-loop is already folded into AT's diag)
            sq_sb = spool.tile([P, n_it], f32)
            nc.scalar.sqrt(sq_sb[:], sT_sb[:])
            d_col = cpool.tile([P, n_it], f32)
            nc.vector.reciprocal(d_col[:], sq_sb[:])

            # AllGather d (tiny), kept in [p, t] layout so the gather back to
            # SBUF reads contiguous 64B runs: d_all[m, p*n_it + t] with global
            # j-tile jt = m*n_it + t.
            nc.sync.dma_start(d_local_dr[:, :], d_col[:])
            nc.gpsimd.collective_compute(
                "AllGather",
                mybir.AluOpType.bypass,
                replica_groups=[list(range(n_cores))],
                ins=[d_local_dr[:, :]],
                outs=[d_all_dr[:, :]],
            )
            d_j = cpool.tile([P, n_jt], f32)
            nc.sync.dma_start(
                d_j[:].rearrange("p (m t) -> p m t", m=n_cores),
                d_all_dr[:, :].rearrange("m (p t) -> p m t", p=P),
            )

            # ---- Phase 2: out = d * (AT^T @ g) ----
            # Two [*, out_dim] accumulation chains share each PSUM bank.
            # Measured HW semantics (psum_probe.py): start=False accumulates
            # unconditionally; start=True overwrites only the region it writes.
            # So zero each whole bank once with a zero-stationary matmul, then
            # run both chains with start=False throughout.
            n_acc = (n_it + 1) // 2
            mm_ps = [
                pspool.tile([P, 2 * out_dim], f32, name=f"mm{q}", tag=f"bank{q}")
                for q in range(n_acc)
            ]
            for q in range(n_acc):
                nc.tensor.matmul(
                    mm_ps[q][:, :],
                    zeros_sb[:],
                    hall_sb[:, : 2 * out_dim],
                    start=True,
                    stop=True,
                    skip_group_check=True,
                )
            for jt in range(n_jt):
                slab = slab_pool.tile([P, rows], bf16, name="slab2", tag="slab")
                nc.sync.dma_start(slab[:], at_d[jt * P : (jt + 1) * P, :])
                # g[jt] = d[jt] * h[jt]  on the (otherwise idle) vector engine
                g_t = spool.tile([P, out_dim], bf16, name="g_t", tag="gt", bufs=6)
                nc.vector.tensor_scalar(
                    g_t[:],
                    hall_sb[:, jt * out_dim : (jt + 1) * out_dim],
                    d_j[:, jt : jt + 1],
                    None,
                    op0=mybir.AluOpType.mult,
                )
                for it in range(n_it):
                    nc.tensor.matmul(
                        mm_ps[it // 2][:, (it % 2) * out_dim : (it % 2 + 1) * out_dim],
                        slab[:, it * P : (it + 1) * P],
                        g_t[:],
                        start=False,
                        stop=(jt == n_jt - 1),
                        skip_group_check=True,
                    )
            for it in range(n_it):
                o_t = spool.tile([P, out_dim], f32, name="o_t", tag="ot", bufs=3)
                nc.vector.tensor_scalar(
                    o_t[:],
                    mm_ps[it // 2][:, (it % 2) * out_dim : (it % 2 + 1) * out_dim],
                    d_col[:, it : it + 1],
                    None,
                    op0=mybir.AluOpType.mult,
                )
                nc.sync.dma_start(out_d[it * P : (it + 1) * P, :], o_t[:])

    nc.compile()
    return nc


def _get_program(key):
    if key not in _PROGRAM_CACHE:
        _PROGRAM_CACHE[key] = build_program(*key)
    return _PROGRAM_CACHE[key]


def make_in_maps(x, adj_mat, W, b, n_total, rows, n_cores):
    """Shard + lay out inputs per core: AT bf16 with +I folded, x^T, W^T, b."""
    x = np.asarray(x, dtype=np.float32)
    adj = np.asarray(adj_mat, dtype=np.float32)
    W = np.asarray(W, dtype=np.float32)
    b = np.asarray(b, dtype=np.float32)

    WT = np.ascontiguousarray(W.T)
    brep = np.ascontiguousarray(np.broadcast_to(b[None, :], (128, W.shape[0])))
    il = np.arange(rows)
    in_maps = []
    for m in range(n_cores):
        sl = slice(m * rows, (m + 1) * rows)
        at_bf = np.ascontiguousarray(adj[sl, :].astype(BF16).T)  # [n_total, rows]
        diag = adj[m * rows + il, m * rows + il] + np.float32(1.0)
        at_bf[m * rows + il, il] = diag.astype(BF16)
        xt = np.ascontiguousarray(x[sl, :].T)
        in_maps.append({"at": at_bf, "xt": xt, "wt": WT, "brep": brep})
    return in_maps


def run_sharded(x, adj_mat, W, b, n_total, in_dim, out_dim, n_cores, **run_kwargs):
    global LAST_RESULTS
    from concourse.bass_utils import run_bass_kernel_spmd

    rows = n_total // n_cores
    nc = _get_program((n_total, rows, in_dim, out_dim, n_cores))
    in_maps = make_in_maps(x, adj_mat, W, b, n_total, rows, n_cores)
    res = run_bass_kernel_spmd(nc, in_maps, core_ids=list(range(n_cores)), **run_kwargs)
    LAST_RESULTS = res
    return np.concatenate([r["out"] for r in res.results], axis=0)


def kernel(x=None, adj_mat=None, W=None, b=None, **_ignored):
    out = run_sharded(x, adj_mat, W, b, N, IN_DIM, OUT_DIM, NCORES)
    return np.asarray(out, dtype=np.float32)


# revision 21
# speedup vs baseline: 1.0938x; 1.0028x over previous
"""GCNConv on 8 Trainium2 NeuronCores.

Computes out = D^-1/2 (A+I) D^-1/2 (x @ W.T + b) for a dense adjacency
A [16384, 16384], x [16384, 512], W [256, 512], b [256].

Strategy (1D row partition over 8 cores, 2048 rows each):
 - Host prep per core m: upload AT_m = (A[rows_m, :] + I)^T as bf16
   [16384, 2048] (partition-natural layout for the PE contraction over j),
   x^T shard fp32, W^T fp32, b replicated.
 - Phase 1 (device): rowsums s_i = sum_j (A+I)[i, j] via ones^T @ AT
   matmuls on the tensor engine; h = x @ W.T + b in fp32 on the PE;
   d = 1/sqrt(s) (ACT sqrt + DVE reciprocal); g = d * h cast to bf16.
 - AllGather g across the 8 cores (2048x256 bf16 per rank -> 16384x256).
 - Phase 2 (device): out_m = d * (AT_m^T @ g) with PSUM accumulation over
   all 128 j-tiles; row-scale by d; write out [2048, 256] fp32.
"""

import numpy as np
import ml_dtypes

N = 16384
IN_DIM = 512
OUT_DIM = 256
NCORES = 8
ROWS = N // NCORES  # 2048

BF16 = ml_dtypes.bfloat16

# Most recent BassKernelResults (for test harnesses that want exec_time_ns).
LAST_RESULTS = None

_PROGRAM_CACHE = {}


def build_program(n_total, rows, in_dim, out_dim, n_cores, slab_bufs=12):
    """Build + compile the per-core Bass program (same NEFF on all cores)."""
    import concourse.bacc as bacc
    import concourse.mybir as mybir
    import concourse.tile as tile

    f32 = mybir.dt.float32
    bf16 = mybir.dt.bfloat16
    P = 128

    n_jt = n_total // P            # tiles along the contraction axis j
    n_it = rows // P               # output row tiles per core
    rs_chunk = min(512, rows)      # rowsum moving free size
    n_rs = rows // rs_chunk        # rowsum chunks (psum banks used)
    n_ct = in_dim // P             # contraction tiles for the h GEMM
    assert n_rs <= 4 and n_it <= 16

    nc = bacc.Bacc(
        "TRN2",
        target_bir_lowering=False,
        debug=False,
        enable_asserts=False,
        num_devices=n_cores,
    )

    at_d = nc.dram_tensor("at", [n_total, rows], bf16, kind="ExternalInput").ap()
    xt_d = nc.dram_tensor("xt", [in_dim, rows], bf16, kind="ExternalInput").ap()
    wt_d = nc.dram_tensor("wt", [in_dim, out_dim], bf16, kind="ExternalInput").ap()
    brep_d = nc.dram_tensor("brep", [P, out_dim], f32, kind="ExternalInput").ap()
    out_d = nc.dram_tensor("out", [rows, out_dim], f32, kind="ExternalOutput").ap()

    with tile.TileContext(nc) as tc:
        with (
            tc.tile_pool(name="const", bufs=1) as cpool,
            tc.tile_pool(name="slab", bufs=slab_bufs) as slab_pool,
            tc.tile_pool(name="small", bufs=2) as spool,
            tc.tile_pool(name="psum", bufs=1, space="PSUM") as pspool,
            tc.tile_pool(name="dram", bufs=1, space="DRAM") as dram_pool,
        ):
            ones_sb = cpool.tile([P, 1], bf16)
            nc.vector.memset(ones_sb[:], 1.0)
            zeros_sb = cpool.tile([P, P], bf16)
            nc.vector.memset(zeros_sb[:], 0.0)

            # x^T staged as [c, ct*rows + i]; W^T as [c, ct*out_dim + k]
            xt_sb = cpool.tile([P, n_ct * rows], bf16)
            nc.gpsimd.dma_start(
                xt_sb[:].rearrange("c (ct i) -> c ct i", ct=n_ct),
                xt_d.rearrange("(ct c) i -> c ct i", c=P),
            )
            wt_sb = cpool.tile([P, n_ct * out_dim], bf16)
            nc.gpsimd.dma_start(
                wt_sb[:].rearrange("c (ct k) -> c ct k", ct=n_ct),
                wt_d.rearrange("(ct c) k -> c ct k", c=P),
            )
            brep_sb = cpool.tile([P, out_dim], f32)
            nc.gpsimd.dma_start(brep_sb[:], brep_d[:, :])

            # bf16 h for the whole graph, laid out [p, jt*out_dim + k]
            hall_sb = cpool.tile([P, n_jt * out_dim], bf16)

            h_local_dr = dram_pool.tile([rows, out_dim], bf16)
            h_all_dr = dram_pool.tile([n_total, out_dim], bf16, addr_space="Shared")
            s_dr = dram_pool.tile([1, rows], f32)
            d_local_dr = dram_pool.tile([P, n_it], f32)
            d_all_dr = dram_pool.tile([n_cores, P * n_it], f32, addr_space="Shared")

            # ---- Phase 1: rowsums of (A+I) via ones^T @ AT, with the small
            # h = x @ W.T + b GEMM interleaved between slab groups (keeps the
            # in-order PE stream dense; each h chain's DVE evacuation overlaps
            # the following rowsum matmuls). AllGather h fires mid-phase.
            s_ps = [
                pspool.tile([1, rs_chunk], f32, name=f"s_ps{c}", tag=f"bank{c}")
                for c in range(n_rs)
            ]

            def emit_h_tile(it):
                h_ps = pspool.tile(
                    [P, out_dim], f32, name="h_ps", tag=f"bank{n_rs + it % 2}"
                )
                for ct in range(n_ct):
                    nc.tensor.matmul(
                        h_ps[:, :],
                        xt_sb[:, ct * rows + it * P : ct * rows + (it + 1) * P],
                        wt_sb[:, ct * out_dim : (ct + 1) * out_dim],
                        start=(ct == 0),
                        stop=(ct == n_ct - 1),
                    )
                hb_t = spool.tile([P, out_dim], bf16, name="hb_t", tag="hbt", bufs=3)
                nc.vector.tensor_add(hb_t[:], h_ps[:, :], brep_sb[:])
                nc.gpsimd.dma_start(h_local_dr[it * P : (it + 1) * P, :], hb_t[:])

            h_stride = max(1, min(4, n_jt // (n_it + 2)))
            h_emitted = 0
            for jt in range(n_jt):
                slab = slab_pool.tile([P, rows], bf16, name="slab", tag="slab")
                nc.sync.dma_start(slab[:], at_d[jt * P : (jt + 1) * P, :])
                for c in range(n_rs):
                    nc.tensor.matmul(
                        s_ps[c][:, :],
                        ones_sb[:],
                        slab[:, c * rs_chunk : (c + 1) * rs_chunk],
                        start=(jt == 0),
                        stop=(jt == n_jt - 1),
                    )
                if jt >= 4 and (jt - 4) % h_stride == 0 and h_emitted < n_it:
                    emit_h_tile(h_emitted)
                    h_emitted += 1
            while h_emitted < n_it:
                emit_h_tile(h_emitted)
                h_emitted += 1

            nc.gpsimd.collective_compute(
                "AllGather",
                mybir.AluOpType.bypass,
                replica_groups=[list(range(n_cores))],
                ins=[h_local_dr[:, :]],
                outs=[h_all_dr[:, :]],
            )

            # h_all -> SBUF, on the gpsimd (SWDGE) queues so it doesn't
            # head-of-line-block the slab stream while waiting on the AG.
            # Chunked to stay under the per-DMA descriptor limit.
            hall_chunks = max(1, n_jt // 16)
            cjt = n_jt // hall_chunks
            for hc in range(hall_chunks):
                nc.gpsimd.dma_start(
                    hall_sb[:, hc * cjt * out_dim : (hc + 1) * cjt * out_dim]
                    .rearrange("p (jt k) -> p jt k", jt=cjt),
                    h_all_dr[hc * cjt * P : (hc + 1) * cjt * P, :]
                    .rearrange("(jt p) k -> p jt k", p=P),
                )


            # s -> SBUF -> DRAM -> per-partition [p, t] layout
            s_sb = spool.tile([1, rows], f32)
            for c in range(n_rs):
                nc.scalar.copy(
                    s_sb[:, c * rs_chunk : (c + 1) * rs_chunk], s_ps[c][:, :]
                )
            nc.sync.dma_start(s_dr[:, :], s_sb[:])
            sT_sb = spool.tile([P, n_it], f32)
            nc.sync.dma_start(sT_sb[:], s_dr[0, :].rearrange("(t p) -> p t", p=P))

            # d = 1/sqrt(s)  (the +1 self-loop is already folded into AT's diag)
            sq_sb = spool.tile([P, n_it], f32)
            nc.scalar.sqrt(sq_sb[:], sT_sb[:])
            d_col = cpool.tile([P, n_it], f32)
            nc.vector.reciprocal(d_col[:], sq_sb[:])

            # AllGather d (tiny), kept in [p, t] layout so the gather back to
            # SBUF reads contiguous 64B runs: d_all[m, p*n_it + t] with global
            # j-tile jt = m*n_it + t.
            nc.sync.dma_start(d_local_dr[:, :], d_col[:])
            nc.gpsimd.collective_compute(
                "AllGather",
                mybir.AluOpType.bypass,
                replica_groups=[list(range(n_cores))],
                ins=[d_local_dr[:, :]],
                outs=[d_all_dr[:, :]],
            )
            d_j = cpool.tile([P, n_jt], f32)
            nc.sync.dma_start(
                d_j[:].rearrange("p (m t) -> p m t", m=n_cores),
                d_all_dr[:, :].rearrange("m (p t) -> p m t", p=P),
            )

            # ---- Phase 2: out = d * (AT^T @ g) ----
            # Two [*, out_dim] accumulation chains share each PSUM bank.
            # Measured HW semantics (psum_probe.py): start=False accumulates
            # unconditionally; start=True overwrites only the region it writes.
            # So zero each whole bank once with a zero-stationary matmul, then
            # run both chains with start=False throughout.
            n_acc = (n_it + 1) // 2
            mm_ps = [
                pspool.tile([P, 2 * out_dim], f32, name=f"mm{q}", tag=f"bank{q}")
                for q in range(n_acc)
            ]
            for q in range(n_acc):
                nc.tensor.matmul(
                    mm_ps[q][:, :],
                    zeros_sb[:],
                    hall_sb[:, : 2 * out_dim],
                    start=True,
                    stop=True,
                    skip_group_check=True,
                )
            for jt in range(n_jt):
                slab = slab_pool.tile([P, rows], bf16, name="slab2", tag="slab")
                nc.sync.dma_start(slab[:], at_d[jt * P : (jt + 1) * P, :])
                # g[jt] = d[jt] * h[jt]  on the (otherwise idle) vector engine
                g_t = spool.tile([P, out_dim], bf16, name="g_t", tag="gt", bufs=6)
                nc.vector.tensor_scalar(
                    g_t[:],
                    hall_sb[:, jt * out_dim : (jt + 1) * out_dim],
                    d_j[:, jt : jt + 1],
                    None,
                    op0=mybir.AluOpType.mult,
                )
                for it in range(n_it):
                    nc.tensor.matmul(
                        mm_ps[it // 2][:, (it % 2) * out_dim : (it % 2 + 1) * out_dim],
                        slab[:, it * P : (it + 1) * P],
                        g_t[:],
                        start=False,
                        stop=(jt == n_jt - 1),
                        skip_group_check=True,
                    )
            for it in range(n_it):
                o_t = spool.tile([P, out_dim], f32, name="o_t", tag="ot", bufs=3)
                nc.vector.tensor_scalar(
                    o_t[:],
                    mm_ps[it // 2][:, (it % 2) * out_dim : (it % 2 + 1) * out_dim],
                    d_col[:, it : it + 1],
                    None,
                    op0=mybir.AluOpType.mult,
                )
                nc.sync.dma_start(out_d[it * P : (it + 1) * P, :], o_t[:])

    nc.compile()
    return nc


def _get_program(key):
    if key not in _PROGRAM_CACHE:
        _PROGRAM_CACHE[key] = build_program(*key)
    return _PROGRAM_CACHE[key]


def make_in_maps(x, adj_mat, W, b, n_total, rows, n_cores):
    """Shard + lay out inputs per core: AT bf16 with +I folded, x^T, W^T, b."""
    x = np.asarray(x, dtype=np.float32)
    adj = np.asarray(adj_mat, dtype=np.float32)
    W = np.asarray(W, dtype=np.float32)
    b = np.asarray(b, dtype=np.float32)

    WT = np.ascontiguousarray(W.T.astype(BF16))
    brep = np.ascontiguousarray(np.broadcast_to(b[None, :], (128, W.shape[0])))
    il = np.arange(rows)
    in_maps = []
    for m in range(n_cores):
        sl = slice(m * rows, (m + 1) * rows)
        at_bf = np.ascontiguousarray(adj[sl, :].astype(BF16).T)  # [n_total, rows]
        diag = adj[m * rows + il, m * rows + il] + np.float32(1.0)
        at_bf[m * rows + il, il] = diag.astype(BF16)
        xt = np.ascontiguousarray(x[sl, :].T.astype(BF16))
        in_maps.append({"at": at_bf, "xt": xt, "wt": WT, "brep": brep})
    return in_maps


def run_sharded(x, adj_mat, W, b, n_total, in_dim, out_dim, n_cores, **run_kwargs):
    global LAST_RESULTS
    from concourse.bass_utils import run_bass_kernel_spmd

    rows = n_total // n_cores
    nc = _get_program((n_total, rows, in_dim, out_dim, n_cores))
    in_maps = make_in_maps(x, adj_mat, W, b, n_total, rows, n_cores)
    res = run_bass_kernel_spmd(nc, in_maps, core_ids=list(range(n_cores)), **run_kwargs)
    LAST_RESULTS = res
    return np.concatenate([r["out"] for r in res.results], axis=0)


def kernel(x=None, adj_mat=None, W=None, b=None, **_ignored):
    out = run_sharded(x, adj_mat, W, b, N, IN_DIM, OUT_DIM, NCORES)
    return np.asarray(out, dtype=np.float32)


# revision 25
# speedup vs baseline: 1.1359x; 1.0385x over previous
"""GCNConv on 8 Trainium2 NeuronCores.

Computes out = D^-1/2 (A+I) D^-1/2 (x @ W.T + b) for a dense adjacency
A [16384, 16384], x [16384, 512], W [256, 512], b [256].

Strategy (1D row partition over 8 cores, 2048 rows each):
 - Host prep per core m: upload AT_m = (A[rows_m, :] + I)^T as bf16
   [16384, 2048] (partition-natural layout for the PE contraction over j),
   x^T shard fp32, W^T fp32, b replicated.
 - Phase 1 (device): rowsums s_i = sum_j (A+I)[i, j] via ones^T @ AT
   matmuls on the tensor engine; h = x @ W.T + b in fp32 on the PE;
   d = 1/sqrt(s) (ACT sqrt + DVE reciprocal); g = d * h cast to bf16.
 - AllGather g across the 8 cores (2048x256 bf16 per rank -> 16384x256).
 - Phase 2 (device): out_m = d * (AT_m^T @ g) with PSUM accumulation over
   all 128 j-tiles; row-scale by d; write out [2048, 256] fp32.
"""

import numpy as np
import ml_dtypes

N = 16384
IN_DIM = 512
OUT_DIM = 256
NCORES = 8
ROWS = N // NCORES  # 2048

BF16 = ml_dtypes.bfloat16

# Most recent BassKernelResults (for test harnesses that want exec_time_ns).
LAST_RESULTS = None

_PROGRAM_CACHE = {}


def build_program(n_total, rows, in_dim, out_dim, n_cores, slab_bufs=12):
    """Build + compile the per-core Bass program (same NEFF on all cores)."""
    import concourse.bacc as bacc
    import concourse.mybir as mybir
    import concourse.tile as tile

    f32 = mybir.dt.float32
    bf16 = mybir.dt.bfloat16
    P = 128

    n_jt = n_total // P            # tiles along the contraction axis j
    n_it = rows // P               # output row tiles per core
    rs_chunk = min(512, rows)      # rowsum moving free size
    n_rs = rows // rs_chunk        # rowsum chunks (psum banks used)
    n_ct = in_dim // P             # contraction tiles for the h GEMM
    assert n_rs <= 4 and n_it <= 16

    nc = bacc.Bacc(
        "TRN2",
        target_bir_lowering=False,
        debug=False,
        enable_asserts=False,
        num_devices=n_cores,
    )

    at_d = nc.dram_tensor("at", [n_total, rows], bf16, kind="ExternalInput").ap()
    xt_d = nc.dram_tensor("xt", [in_dim, rows], bf16, kind="ExternalInput").ap()
    wt_d = nc.dram_tensor("wt", [in_dim, out_dim], bf16, kind="ExternalInput").ap()
    brep_d = nc.dram_tensor("brep", [P, out_dim], f32, kind="ExternalInput").ap()
    out_d = nc.dram_tensor("out", [rows, out_dim], f32, kind="ExternalOutput").ap()

    with tile.TileContext(nc) as tc:
        with (
            tc.tile_pool(name="const", bufs=1) as cpool,
            tc.tile_pool(name="slab", bufs=slab_bufs) as slab_pool,
            tc.tile_pool(name="small", bufs=2) as spool,
            tc.tile_pool(name="psum", bufs=1, space="PSUM") as pspool,
            tc.tile_pool(name="dram", bufs=1, space="DRAM") as dram_pool,
        ):
            ones_sb = cpool.tile([P, 1], bf16)
            nc.vector.memset(ones_sb[:], 1.0)
            zeros_sb = cpool.tile([P, P], bf16)
            nc.vector.memset(zeros_sb[:], 0.0)

            # x^T staged as [c, ct*rows + i]; W^T as [c, ct*out_dim + k]
            xt_sb = cpool.tile([P, n_ct * rows], bf16)
            nc.gpsimd.dma_start(
                xt_sb[:].rearrange("c (ct i) -> c ct i", ct=n_ct),
                xt_d.rearrange("(ct c) i -> c ct i", c=P),
            )
            wt_sb = cpool.tile([P, n_ct * out_dim], bf16)
            nc.gpsimd.dma_start(
                wt_sb[:].rearrange("c (ct k) -> c ct k", ct=n_ct),
                wt_d.rearrange("(ct c) k -> c ct k", c=P),
            )
            brep_sb = cpool.tile([P, out_dim], f32)
            nc.gpsimd.dma_start(brep_sb[:], brep_d[:, :])

            # bf16 h for the whole graph, laid out [p, jt*out_dim + k]
            hall_sb = cpool.tile([P, n_jt * out_dim], bf16)

            h_local_dr = dram_pool.tile([rows, out_dim], bf16)
            h_all_dr = dram_pool.tile([n_total, out_dim], bf16, addr_space="Shared")
            s_dr = dram_pool.tile([1, rows], f32)
            d_local_dr = dram_pool.tile([P, n_it], f32)
            d_all_dr = dram_pool.tile([n_cores, P * n_it], f32, addr_space="Shared")

            # ---- Phase 1: rowsums of (A+I) via ones^T @ AT, with the small
            # h = x @ W.T + b GEMM interleaved between slab groups (keeps the
            # in-order PE stream dense; each h chain's DVE evacuation overlaps
            # the following rowsum matmuls). AllGather h fires mid-phase.
            s_ps = [
                pspool.tile([1, rs_chunk], f32, name=f"s_ps{c}", tag=f"bank{c}")
                for c in range(n_rs)
            ]

            def emit_h_tile(it):
                h_ps = pspool.tile(
                    [P, out_dim], f32, name="h_ps", tag=f"bank{n_rs + it % 4}"
                )
                for ct in range(n_ct):
                    nc.tensor.matmul(
                        h_ps[:, :],
                        xt_sb[:, ct * rows + it * P : ct * rows + (it + 1) * P],
                        wt_sb[:, ct * out_dim : (ct + 1) * out_dim],
                        start=(ct == 0),
                        stop=(ct == n_ct - 1),
                    )
                hb_t = spool.tile([P, out_dim], bf16, name="hb_t", tag="hbt", bufs=3)
                nc.vector.tensor_add(hb_t[:], h_ps[:, :], brep_sb[:])
                nc.gpsimd.dma_start(h_local_dr[it * P : (it + 1) * P, :], hb_t[:])

            h_stride = max(1, min(4, n_jt // (n_it + 2)))
            h_emitted = 0
            last_slab = None
            for jt in range(n_jt):
                slab = slab_pool.tile([P, rows], bf16, name="slab", tag="slab")
                last_slab = slab
                nc.sync.dma_start(slab[:], at_d[jt * P : (jt + 1) * P, :])
                for c in range(n_rs):
                    nc.tensor.matmul(
                        s_ps[c][:, :],
                        ones_sb[:],
                        slab[:, c * rs_chunk : (c + 1) * rs_chunk],
                        start=(jt == 0),
                        stop=(jt == n_jt - 1),
                    )
                if jt >= 4 and (jt - 4) % h_stride == 0 and h_emitted < n_it:
                    emit_h_tile(h_emitted)
                    h_emitted += 1
            while h_emitted < n_it:
                emit_h_tile(h_emitted)
                h_emitted += 1

            nc.gpsimd.collective_compute(
                "AllGather",
                mybir.AluOpType.bypass,
                replica_groups=[list(range(n_cores))],
                ins=[h_local_dr[:, :]],
                outs=[h_all_dr[:, :]],
            )

            # h_all -> SBUF, on the gpsimd (SWDGE) queues so it doesn't
            # head-of-line-block the slab stream while waiting on the AG.
            # Chunked to stay under the per-DMA descriptor limit.
            hall_chunks = max(1, n_jt // 16)
            cjt = n_jt // hall_chunks
            for hc in range(hall_chunks):
                nc.gpsimd.dma_start(
                    hall_sb[:, hc * cjt * out_dim : (hc + 1) * cjt * out_dim]
                    .rearrange("p (jt k) -> p jt k", jt=cjt),
                    h_all_dr[hc * cjt * P : (hc + 1) * cjt * P, :]
                    .rearrange("(jt p) k -> p jt k", p=P),
                )


            # s -> SBUF -> DRAM -> per-partition [p, t] layout
            s_sb = spool.tile([1, rows], f32)
            for c in range(n_rs):
                nc.scalar.copy(
                    s_sb[:, c * rs_chunk : (c + 1) * rs_chunk], s_ps[c][:, :]
                )
            nc.sync.dma_start(s_dr[:, :], s_sb[:])
            sT_sb = spool.tile([P, n_it], f32)
            nc.sync.dma_start(sT_sb[:], s_dr[0, :].rearrange("(t p) -> p t", p=P))

            # d = 1/sqrt(s)  (the +1 self-loop is already folded into AT's diag)
            sq_sb = spool.tile([P, n_it], f32)
            nc.scalar.sqrt(sq_sb[:], sT_sb[:])
            d_col = cpool.tile([P, n_it], f32)
            nc.vector.reciprocal(d_col[:], sq_sb[:])

            # AllGather d (tiny), kept in [p, t] layout so the gather back to
            # SBUF reads contiguous 64B runs: d_all[m, p*n_it + t] with global
            # j-tile jt = m*n_it + t.
            nc.sync.dma_start(d_local_dr[:, :], d_col[:])
            nc.gpsimd.collective_compute(
                "AllGather",
                mybir.AluOpType.bypass,
                replica_groups=[list(range(n_cores))],
                ins=[d_local_dr[:, :]],
                outs=[d_all_dr[:, :]],
            )
            d_j = cpool.tile([P, n_jt], f32)
            nc.sync.dma_start(
                d_j[:].rearrange("p (m t) -> p m t", m=n_cores),
                d_all_dr[:, :].rearrange("m (p t) -> p m t", p=P),
            )

            # ---- Phase 2: out = d * (AT^T @ g) ----
            # Two [*, out_dim] accumulation chains share each PSUM bank.
            # Measured HW semantics (psum_probe.py): start=False accumulates
            # unconditionally; start=True overwrites only the region it writes.
            # So zero each whole bank once with a zero-stationary matmul, then
            # run both chains with start=False throughout.
            n_acc = (n_it + 1) // 2
            mm_ps = [
                pspool.tile([P, 2 * out_dim], f32, name=f"mm{q}", tag=f"bank{q}")
                for q in range(n_acc)
            ]
            # rhs = the final phase-1 slab, so the scheduler cannot hoist these
            # into the middle of the in-order PE stream (they'd block on the
            # h AllGather and stall the remaining rowsum matmuls behind them).
            dw = min(2 * out_dim, rows)
            for q in range(n_acc):
                for off in range(0, 2 * out_dim, dw):
                    nc.tensor.matmul(
                        mm_ps[q][:, off : off + dw],
                        zeros_sb[:],
                        last_slab[:, :dw],
                        start=True,
                        stop=True,
                        skip_group_check=True,
                    )
            for jt in range(n_jt):
                slab = slab_pool.tile([P, rows], bf16, name="slab2", tag="slab")
                nc.sync.dma_start(slab[:], at_d[jt * P : (jt + 1) * P, :])
                # g[jt] = d[jt] * h[jt]  on the (otherwise idle) vector engine
                g_t = spool.tile([P, out_dim], bf16, name="g_t", tag="gt", bufs=6)
                nc.vector.tensor_scalar(
                    g_t[:],
                    hall_sb[:, jt * out_dim : (jt + 1) * out_dim],
                    d_j[:, jt : jt + 1],
                    None,
                    op0=mybir.AluOpType.mult,
                )
                for it in range(n_it):
                    nc.tensor.matmul(
                        mm_ps[it // 2][:, (it % 2) * out_dim : (it % 2 + 1) * out_dim],
                        slab[:, it * P : (it + 1) * P],
                        g_t[:],
                        start=False,
                        stop=(jt == n_jt - 1),
                        skip_group_check=True,
                    )
            for it in range(n_it):
                o_t = spool.tile([P, out_dim], f32, name="o_t", tag="ot", bufs=3)
                nc.vector.tensor_scalar(
                    o_t[:],
                    mm_ps[it // 2][:, (it % 2) * out_dim : (it % 2 + 1) * out_dim],
                    d_col[:, it : it + 1],
                    None,
                    op0=mybir.AluOpType.mult,
                )
                nc.sync.dma_start(out_d[it * P : (it + 1) * P, :], o_t[:])

    nc.compile()
    return nc


def _get_program(key):
    if key not in _PROGRAM_CACHE:
        _PROGRAM_CACHE[key] = build_program(*key)
    return _PROGRAM_CACHE[key]


def make_in_maps(x, adj_mat, W, b, n_total, rows, n_cores):
    """Shard + lay out inputs per core: AT bf16 with +I folded, x^T, W^T, b."""
    x = np.asarray(x, dtype=np.float32)
    adj = np.asarray(adj_mat, dtype=np.float32)
    W = np.asarray(W, dtype=np.float32)
    b = np.asarray(b, dtype=np.float32)

    WT = np.ascontiguousarray(W.T.astype(BF16))
    brep = np.ascontiguousarray(np.broadcast_to(b[None, :], (128, W.shape[0])))
    il = np.arange(rows)
    in_maps = []
    for m in range(n_cores):
        sl = slice(m * rows, (m + 1) * rows)
        at_bf = np.ascontiguousarray(adj[sl, :].astype(BF16).T)  # [n_total, rows]
        diag = adj[m * rows + il, m * rows + il] + np.float32(1.0)
        at_bf[m * rows + il, il] = diag.astype(BF16)
        xt = np.ascontiguousarray(x[sl, :].T.astype(BF16))
        in_maps.append({"at": at_bf, "xt": xt, "wt": WT, "brep": brep})
    return in_maps


def run_sharded(x, adj_mat, W, b, n_total, in_dim, out_dim, n_cores, **run_kwargs):
    global LAST_RESULTS
    from concourse.bass_utils import run_bass_kernel_spmd

    rows = n_total // n_cores
    nc = _get_program((n_total, rows, in_dim, out_dim, n_cores))
    in_maps = make_in_maps(x, adj_mat, W, b, n_total, rows, n_cores)
    res = run_bass_kernel_spmd(nc, in_maps, core_ids=list(range(n_cores)), **run_kwargs)
    LAST_RESULTS = res
    return np.concatenate([r["out"] for r in res.results], axis=0)


def kernel(x=None, adj_mat=None, W=None, b=None, **_ignored):
    out = run_sharded(x, adj_mat, W, b, N, IN_DIM, OUT_DIM, NCORES)
    return np.asarray(out, dtype=np.float32)


# revision 29
# speedup vs baseline: 1.1906x; 1.0482x over previous
"""GCNConv on 8 Trainium2 NeuronCores.

Computes out = D^-1/2 (A+I) D^-1/2 (x @ W.T + b) for a dense adjacency
A [16384, 16384], x [16384, 512], W [256, 512], b [256].

Strategy (1D row partition over 8 cores, 2048 rows each):
 - Host prep per core m: upload AT_m = (A[rows_m, :] + I)^T as bf16
   [16384, 2048] (partition-natural layout for the PE contraction over j),
   x^T shard fp32, W^T fp32, b replicated.
 - Phase 1 (device): rowsums s_i = sum_j (A+I)[i, j] via ones^T @ AT
   matmuls on the tensor engine; h = x @ W.T + b in fp32 on the PE;
   d = 1/sqrt(s) (ACT sqrt + DVE reciprocal); g = d * h cast to bf16.
 - AllGather g across the 8 cores (2048x256 bf16 per rank -> 16384x256).
 - Phase 2 (device): out_m = d * (AT_m^T @ g) with PSUM accumulation over
   all 128 j-tiles; row-scale by d; write out [2048, 256] fp32.
"""

import numpy as np
import ml_dtypes

N = 16384
IN_DIM = 512
OUT_DIM = 256
NCORES = 8
ROWS = N // NCORES  # 2048

BF16 = ml_dtypes.bfloat16

# Most recent BassKernelResults (for test harnesses that want exec_time_ns).
LAST_RESULTS = None

_PROGRAM_CACHE = {}


def build_program(n_total, rows, in_dim, out_dim, n_cores, slab_bufs=12):
    """Build + compile the per-core Bass program (same NEFF on all cores)."""
    import concourse.bacc as bacc
    import concourse.mybir as mybir
    import concourse.tile as tile

    f32 = mybir.dt.float32
    bf16 = mybir.dt.bfloat16
    P = 128

    n_jt = n_total // P            # tiles along the contraction axis j
    n_it = rows // P               # output row tiles per core
    rs_chunk = min(512, rows)      # rowsum moving free size
    n_rs = rows // rs_chunk        # rowsum chunks (psum banks used)
    n_ct = in_dim // P             # contraction tiles for the h GEMM
    assert n_rs <= 4 and n_it <= 16

    nc = bacc.Bacc(
        "TRN2",
        target_bir_lowering=False,
        debug=False,
        enable_asserts=False,
        num_devices=n_cores,
    )

    at_d = nc.dram_tensor("at", [n_total, rows], bf16, kind="ExternalInput").ap()
    xt_d = nc.dram_tensor("xt", [in_dim, rows], bf16, kind="ExternalInput").ap()
    wt_d = nc.dram_tensor("wt", [in_dim, out_dim], bf16, kind="ExternalInput").ap()
    brep_d = nc.dram_tensor("brep", [P, out_dim], f32, kind="ExternalInput").ap()
    out_d = nc.dram_tensor("out", [rows, out_dim], f32, kind="ExternalOutput").ap()

    with tile.TileContext(nc) as tc:
        with (
            tc.tile_pool(name="const", bufs=1) as cpool,
            tc.tile_pool(name="slab", bufs=slab_bufs) as slab_pool,
            tc.tile_pool(name="small", bufs=2) as spool,
            tc.tile_pool(name="psum", bufs=1, space="PSUM") as pspool,
            tc.tile_pool(name="dram", bufs=1, space="DRAM") as dram_pool,
        ):
            ones_sb = cpool.tile([P, 1], bf16)
            nc.vector.memset(ones_sb[:], 1.0)
            zeros_sb = cpool.tile([P, P], bf16)
            nc.vector.memset(zeros_sb[:], 0.0)

            # x^T staged as [c, ct*rows + i]; W^T as [c, ct*out_dim + k]
            xt_sb = cpool.tile([P, n_ct * rows], bf16)
            nc.gpsimd.dma_start(
                xt_sb[:].rearrange("c (ct i) -> c ct i", ct=n_ct),
                xt_d.rearrange("(ct c) i -> c ct i", c=P),
            )
            wt_sb = cpool.tile([P, n_ct * out_dim], bf16)
            nc.gpsimd.dma_start(
                wt_sb[:].rearrange("c (ct k) -> c ct k", ct=n_ct),
                wt_d.rearrange("(ct c) k -> c ct k", c=P),
            )
            brep_sb = cpool.tile([P, out_dim], f32)
            nc.gpsimd.dma_start(brep_sb[:], brep_d[:, :])

            h_local_dr = dram_pool.tile([rows, out_dim], bf16)
            h_all_dr = dram_pool.tile([n_total, out_dim], bf16, addr_space="Shared")
            s_dr = dram_pool.tile([1, rows], f32)
            d_local_dr = dram_pool.tile([P, n_it], f32)
            d_all_dr = dram_pool.tile([n_cores, P * n_it], f32, addr_space="Shared")

            # ---- Phase 1: rowsums of (A+I) via ones^T @ AT, with the small
            # h = x @ W.T + b GEMM interleaved between slab groups (keeps the
            # in-order PE stream dense; each h chain's DVE evacuation overlaps
            # the following rowsum matmuls). AllGather h fires mid-phase.
            s_ps = [
                pspool.tile([1, rs_chunk], f32, name=f"s_ps{c}", tag=f"bank{c}")
                for c in range(n_rs)
            ]

            def emit_h_tile(it):
                h_ps = pspool.tile(
                    [P, out_dim], f32, name="h_ps", tag=f"bank{n_rs + it % 4}"
                )
                for ct in range(n_ct):
                    nc.tensor.matmul(
                        h_ps[:, :],
                        xt_sb[:, ct * rows + it * P : ct * rows + (it + 1) * P],
                        wt_sb[:, ct * out_dim : (ct + 1) * out_dim],
                        start=(ct == 0),
                        stop=(ct == n_ct - 1),
                    )
                hb_t = spool.tile([P, out_dim], bf16, name="hb_t", tag="hbt", bufs=3)
                nc.vector.tensor_add(hb_t[:], h_ps[:, :], brep_sb[:])
                nc.gpsimd.dma_start(h_local_dr[it * P : (it + 1) * P, :], hb_t[:])

            # Slabs are fetched in pairs of j-tiles (1 MiB per dma_start) to
            # amortize DMA trigger overhead: pair tile [p, u*rows + i] holds
            # j-tiles 2*jp+u.
            n_jp = n_jt // 2
            h_stride = max(1, min(2, n_jp // (n_it + 2)))
            h_emitted = 0
            last_slab = None
            for jp in range(n_jp):
                slab = slab_pool.tile([P, 2 * rows], bf16, name="slab", tag="slab")
                last_slab = slab
                nc.sync.dma_start(
                    slab[:].rearrange("p (u i) -> p u i", u=2),
                    at_d[jp * 2 * P : (jp + 1) * 2 * P, :].rearrange(
                        "(u p) i -> p u i", p=P
                    ),
                )
                for u in range(2):
                    jt = 2 * jp + u
                    for c in range(n_rs):
                        nc.tensor.matmul(
                            s_ps[c][:, :],
                            ones_sb[:],
                            slab[:, u * rows + c * rs_chunk : u * rows + (c + 1) * rs_chunk],
                            start=(jt == 0),
                            stop=(jt == n_jt - 1),
                        )
                if jp >= 2 and (jp - 2) % h_stride == 0 and h_emitted < n_it:
                    emit_h_tile(h_emitted)
                    h_emitted += 1
            while h_emitted < n_it:
                emit_h_tile(h_emitted)
                h_emitted += 1

            nc.gpsimd.collective_compute(
                "AllGather",
                mybir.AluOpType.bypass,
                replica_groups=[list(range(n_cores))],
                ins=[h_local_dr[:, :]],
                outs=[h_all_dr[:, :]],
            )

            # s -> SBUF -> DRAM -> per-partition [p, t] layout
            s_sb = spool.tile([1, rows], f32)
            for c in range(n_rs):
                nc.scalar.copy(
                    s_sb[:, c * rs_chunk : (c + 1) * rs_chunk], s_ps[c][:, :]
                )
            nc.sync.dma_start(s_dr[:, :], s_sb[:])
            sT_sb = spool.tile([P, n_it], f32)
            for tq in range(0, n_it, 4):
                tw = min(4, n_it - tq)
                nc.sync.dma_start(
                    sT_sb[:, tq : tq + tw],
                    s_dr[0, tq * P : (tq + tw) * P].rearrange("(t p) -> p t", p=P),
                )

            # d = 1/sqrt(s)  (the +1 self-loop is already folded into AT's diag)
            sq_sb = spool.tile([P, n_it], f32)
            nc.scalar.sqrt(sq_sb[:], sT_sb[:])
            d_col = cpool.tile([P, n_it], f32)
            nc.vector.reciprocal(d_col[:], sq_sb[:])

            # AllGather d (tiny), kept in [p, t] layout so the gather back to
            # SBUF reads contiguous 64B runs: d_all[m, p*n_it + t] with global
            # j-tile jt = m*n_it + t.
            nc.sync.dma_start(d_local_dr[:, :], d_col[:])
            nc.gpsimd.collective_compute(
                "AllGather",
                mybir.AluOpType.bypass,
                replica_groups=[list(range(n_cores))],
                ins=[d_local_dr[:, :]],
                outs=[d_all_dr[:, :]],
            )
            d_j = cpool.tile([P, n_jt], f32)
            nc.sync.dma_start(
                d_j[:].rearrange("p (m t) -> p m t", m=n_cores),
                d_all_dr[:, :].rearrange("m (p t) -> p m t", p=P),
            )

            # ---- Phase 2: out = d * (AT^T @ g) ----
            # Two [*, out_dim] accumulation chains share each PSUM bank.
            # Measured HW semantics (psum_probe.py): start=False accumulates
            # unconditionally; start=True overwrites only the region it writes.
            # So zero each whole bank once with a zero-stationary matmul, then
            # run both chains with start=False throughout.
            n_acc = (n_it + 1) // 2
            mm_ps = [
                pspool.tile([P, 2 * out_dim], f32, name=f"mm{q}", tag=f"bank{q}")
                for q in range(n_acc)
            ]
            # rhs = the final phase-1 slab, so the scheduler cannot hoist these
            # into the middle of the in-order PE stream (they'd block on the
            # h AllGather and stall the remaining rowsum matmuls behind them).
            dw = min(2 * out_dim, 2 * rows)
            for q in range(n_acc):
                for off in range(0, 2 * out_dim, dw):
                    nc.tensor.matmul(
                        mm_ps[q][:, off : off + dw],
                        zeros_sb[:],
                        last_slab[:, :dw],
                        start=True,
                        stop=True,
                        skip_group_check=True,
                    )
            for jp in range(n_jp):
                slab = slab_pool.tile([P, 2 * rows], bf16, name="slab2", tag="slab")
                nc.sync.dma_start(
                    slab[:].rearrange("p (u i) -> p u i", u=2),
                    at_d[jp * 2 * P : (jp + 1) * 2 * P, :].rearrange(
                        "(u p) i -> p u i", p=P
                    ),
                )
                for u in range(2):
                    jt = 2 * jp + u
                    # h[jt] straight from the gathered DRAM copy (phase-2 DMA
                    # has slack; this keeps the 8 MB out of phase 1), then
                    # g[jt] = d[jt] * h[jt] on the otherwise-idle vector engine
                    hj_t = spool.tile([P, out_dim], bf16, name="hj_t", tag="hjt", bufs=6)
                    nc.sync.dma_start(hj_t[:], h_all_dr[jt * P : (jt + 1) * P, :])
                    g_t = spool.tile([P, out_dim], bf16, name="g_t", tag="gt", bufs=6)
                    nc.vector.tensor_scalar(
                        g_t[:],
                        hj_t[:],
                        d_j[:, jt : jt + 1],
                        None,
                        op0=mybir.AluOpType.mult,
                    )
                    for it in range(n_it):
                        nc.tensor.matmul(
                            mm_ps[it // 2][:, (it % 2) * out_dim : (it % 2 + 1) * out_dim],
                            slab[:, u * rows + it * P : u * rows + (it + 1) * P],
                            g_t[:],
                            start=False,
                            stop=(jt == n_jt - 1),
                            skip_group_check=True,
                        )
            for it in range(n_it):
                o_t = spool.tile([P, out_dim], f32, name="o_t", tag="ot", bufs=3)
                nc.vector.tensor_scalar(
                    o_t[:],
                    mm_ps[it // 2][:, (it % 2) * out_dim : (it % 2 + 1) * out_dim],
                    d_col[:, it : it + 1],
                    None,
                    op0=mybir.AluOpType.mult,
                )
                nc.sync.dma_start(out_d[it * P : (it + 1) * P, :], o_t[:])

    nc.compile()
    return nc


def _get_program(key):
    if key not in _PROGRAM_CACHE:
        _PROGRAM_CACHE[key] = build_program(*key)
    return _PROGRAM_CACHE[key]


def make_in_maps(x, adj_mat, W, b, n_total, rows, n_cores):
    """Shard + lay out inputs per core: AT bf16 with +I folded, x^T, W^T, b."""
    x = np.asarray(x, dtype=np.float32)
    adj = np.asarray(adj_mat, dtype=np.float32)
    W = np.asarray(W, dtype=np.float32)
    b = np.asarray(b, dtype=np.float32)

    WT = np.ascontiguousarray(W.T.astype(BF16))
    brep = np.ascontiguousarray(np.broadcast_to(b[None, :], (128, W.shape[0])))
    il = np.arange(rows)
    in_maps = []
    for m in range(n_cores):
        sl = slice(m * rows, (m + 1) * rows)
        at_bf = np.ascontiguousarray(adj[sl, :].astype(BF16).T)  # [n_total, rows]
        diag = adj[m * rows + il, m * rows + il] + np.float32(1.0)
        at_bf[m * rows + il, il] = diag.astype(BF16)
        xt = np.ascontiguousarray(x[sl, :].T.astype(BF16))
        in_maps.append({"at": at_bf, "xt": xt, "wt": WT, "brep": brep})
    return in_maps


def run_sharded(x, adj_mat, W, b, n_total, in_dim, out_dim, n_cores, **run_kwargs):
    global LAST_RESULTS
    from concourse.bass_utils import run_bass_kernel_spmd

    rows = n_total // n_cores
    nc = _get_program((n_total, rows, in_dim, out_dim, n_cores))
    in_maps = make_in_maps(x, adj_mat, W, b, n_total, rows, n_cores)
    res = run_bass_kernel_spmd(nc, in_maps, core_ids=list(range(n_cores)), **run_kwargs)
    LAST_RESULTS = res
    return np.concatenate([r["out"] for r in res.results], axis=0)


def kernel(x=None, adj_mat=None, W=None, b=None, **_ignored):
    out = run_sharded(x, adj_mat, W, b, N, IN_DIM, OUT_DIM, NCORES)
    return np.asarray(out, dtype=np.float32)
